# revision 2
# baseline (speedup 1.0000x reference)
"""Trainium2 Bass kernel for nn_CrossAttention (b=4, lq=lkv=2048, dq=1024, dkv=768, 4 heads).

Sharding: 8 cores = (batch b in 0..3) x (head-group g in 0..1); each core handles
one batch and 2 of the 4 heads (512 of the 1024 head dims).  All activations are
fed to the device pre-transposed ([model_dim, seq]) so every matmul contracts
over the partition dimension with zero on-device transposes:

  qhT  [512,2048] = WqT.T @ qT          (proj, contraction over dq=1024)
  khT  [512,2048] = WkT.T @ kvT         (proj, contraction over dkv=768)
  vh   [2048,512] = kvT_chunk.T @ WvT   (proj, natural layout)
  sT   [2048,2048] per head = khT_h.T @ qhT_h    (scoresT: lkv on partitions)
  eT   = exp(sT / 16 - 4)               (shift instead of max-subtraction keeps
                                         eT and its fp16 tree sums in fp16 range;
                                         the shift cancels in softmax)
  ctxT [256,2048] per head accumulated over lkv tiles (lhsT=vh, rhs=eT)
  sum  via DVE fp16 add-tree over eT tiles + one ones[128,128] matmul
        (every psum partition gets the column sum -> reciprocal)
  ctxT normalized by DVE mul with the reciprocal tile; the normalization
        tail for chunk i is emitted inside chunk i+1 so PE never stalls
  outT [1024,2048] = WoT.T @ ctxT       (output proj over the core's 512 dims)

All matmul operands are fp16 (full rate, 2-byte LDWEIGHTS, half DMA traffic).

Perf structure (v2):
- All DRAM inputs are packed host-side into [128, X] SBUF-row layout so every
  dma_start is a single 2D descriptor block (~620ns issue instead of ~600ns
  per kt-slice).  Input DMAs are split across the Sync and Scalar (Activation)
  HWDGE queues so issue bandwidth doubles and phase A never starves.
- A few warmup matmuls on zeros run during the initial DMA wait to ramp the
  PE clock out of its low pstate before the first real matmul.
- Phase B computes scores for TWO lkv k-tiles into one 2-bank [128,2,512]
  PSUM tile and runs ONE 1024-wide Exp activation over both banks, halving
  the scalar-engine instruction count; the scalar Exp chain was the phase B
  pipeline limiter (688ns ACT + sync vs 853ns of PE work per k-tile).
- ctx PSUM accumulators are evacuated to SBUF (DVE copies) right at chunk
  end so ps_ctx only needs 3 banks: 4 (scores) + 3 (ctx) + 1 (sum) = 8.
Measured accuracy ~2e-3 max-rel (threshold 2e-2).
Host gathers: out[b] = (outT[core 2b] + outT[core 2b+1]).T + bo.
"""

import numpy as np

B = 4
LQ = 2048
LKV = 2048
DQ = 1024
DKV = 768
HD = 256  # per-head dim
GH = 512  # head dims per core (2 heads)
P = 128
NCORES = 8
NQ = LQ // 512  # lq chunks of 512
KT_Q = DQ // P  # 8
KT_KV = DKV // P  # 6
KT_L = LKV // P  # 16
NKP = KT_L // 2  # 8 lkv k-tile pairs

ESHIFT = -4.0  # exp(s/16 + ESHIFT); cancels in softmax, keeps fp16 in range
NWARM = 8  # PE warmup matmuls during initial DMA wait (pstate ramp)
TRACE = False

_COMPILED = None
last_exec_time_ns = None
last_profile = None


def _emit(tc, aps):
    from contextlib import ExitStack

    import concourse.mybir as mybir

    nc = tc.nc
    f32 = mybir.dt.float32
    dt16 = mybir.dt.float16
    Exp = mybir.ActivationFunctionType.Exp

    qPK, kvPK, wqPK, wkPK, wvPK, woPK, outT = (
        aps["qPK"], aps["kvPK"], aps["wqPK"], aps["wkPK"], aps["wvPK"],
        aps["woPK"], aps["outT"],
    )

    with ExitStack() as top:
        # persistent SBUF tensors
        khT_pool = top.enter_context(tc.tile_pool(name="khT", bufs=1))
        qhT_pool = top.enter_context(tc.tile_pool(name="qhT", bufs=1))
        vh_pool = top.enter_context(tc.tile_pool(name="vh", bufs=1))
        const_pool = top.enter_context(tc.tile_pool(name="const", bufs=1))
        wo_pool = top.enter_context(tc.tile_pool(name="wo", bufs=1))

        khT = [khT_pool.tile([P, LKV], dt16, tag=f"khT{i}", name=f"khT{i}")
               for i in range(4)]
        qhT = [qhT_pool.tile([P, LQ], dt16, tag=f"qhT{i}", name=f"qhT{i}")
               for i in range(4)]
        vh = [vh_pool.tile([P, GH], dt16, tag=f"vh{i}", name=f"vh{i}")
              for i in range(KT_L)]
        wo_t = wo_pool.tile([P, 4, DQ], dt16, tag="wo", name="wo")

        ones_sq = const_pool.tile([P, P], dt16, tag="ones_sq", name="ones_sq")
        ones_f32 = const_pool.tile([P, P], f32, tag="ones_f32", name="ones_f32")
        nc.vector.memset(ones_f32[:], 1.0)
        nc.vector.tensor_copy(ones_sq[:], ones_f32[:])
        ebias = const_pool.tile([P, 1], f32, tag="ebias", name="ebias")
        nc.vector.memset(ebias[:], ESHIFT)
        warm = const_pool.tile([P, 512], dt16, tag="warm", name="warm")
        nc.vector.memset(warm[:], 0.0)

        # ---------------- Phase A: projections ----------------
        with ExitStack() as ph:
            w_pool = ph.enter_context(tc.tile_pool(name="w", bufs=1))
            kvc_pool = ph.enter_context(tc.tile_pool(name="kvc", bufs=4))
            qc_pool = ph.enter_context(tc.tile_pool(name="qc", bufs=4))
            psA = ph.enter_context(tc.tile_pool(name="psA", bufs=4, space="PSUM"))

            # PE warmup on zeros: ramps the clock during the DMA head so the
            # first real matmuls don't run at the low pstate.
            for _ in range(NWARM):
                pw = psA.tile([P, 512], f32, tag="psA", name="pwarm")
                nc.tensor.matmul(pw[:], lhsT=ones_sq[:], rhs=warm[:],
                                 start=True, stop=True)

            wk_t = w_pool.tile([P, KT_KV, GH], dt16, tag="wk", name="wk")
            wv_t = w_pool.tile([P, KT_KV, GH], dt16, tag="wv", name="wv")
            wq_t = w_pool.tile([P, KT_Q, GH], dt16, tag="wq", name="wq")
            kvc0 = kvc_pool.tile([P, KT_KV, 512], dt16, tag="kvc", name="kvc")

            # first-use tiles per-kt interleaved across BOTH hwdge queues so
            # the first vh matmuls start as early as possible
            for kt in range(KT_KV):
                nc.sync.dma_start(wv_t[:, kt, :], wvPK[:, kt * GH:(kt + 1) * GH])
                nc.scalar.dma_start(kvc0[:, kt, :],
                                    kvPK[:, kt * 512:(kt + 1) * 512])
            # rest: one single-block DMA per tile, spread across queues
            nc.sync.dma_start(wk_t[:], wkPK[:])
            kvc_tiles = {0: kvc0}
            for n in range(1, NQ):
                t = kvc_pool.tile([P, KT_KV, 512], dt16, tag="kvc", name="kvc")
                nc.sync.dma_start(t[:], kvPK[:, n * 3072:(n + 1) * 3072])
                kvc_tiles[n] = t
            nc.scalar.dma_start(wq_t[:], wqPK[:])
            qc_tiles = {}
            for n in range(NQ):
                t = qc_pool.tile([P, KT_Q, 512], dt16, tag="qc", name="qc")
                nc.scalar.dma_start(t[:], qPK[:, n * 4096:(n + 1) * 4096])
                qc_tiles[n] = t
            nc.scalar.dma_start(wo_t[:], woPK[:])

            def emit_vh(n):
                # kt-outer so the first matmuls only need the first kt-slice
                # of the chunk (lets compute start while the DMA streams in)
                kvc = kvc_tiles[n]
                ps = [psA.tile([P, 512], f32, tag="psA", name="psA")
                      for _ in range(4)]
                for kt in range(KT_KV):
                    for lj in range(4):
                        nc.tensor.matmul(
                            ps[lj][:],
                            lhsT=kvc[:, kt, lj * P:(lj + 1) * P],
                            rhs=wv_t[:, kt, :],
                            start=(kt == 0),
                            stop=(kt == KT_KV - 1),
                        )
                for lj in range(4):
                    nc.vector.tensor_copy(vh[4 * n + lj][:], ps[lj][:])

            def emit_khT(np_):
                # one stationary load feeds both 512-chunks of the pair
                n0, n1 = 2 * np_, 2 * np_ + 1
                for m in range(4):
                    ps0 = psA.tile([P, 512], f32, tag="psA", name="psA")
                    ps1 = psA.tile([P, 512], f32, tag="psA", name="psA")
                    for kt in range(KT_KV):
                        nc.tensor.matmul(
                            ps0[:],
                            lhsT=wk_t[:, kt, m * P:(m + 1) * P],
                            rhs=kvc_tiles[n0][:, kt, :],
                            start=(kt == 0),
                            stop=(kt == KT_KV - 1),
                        )
                        nc.tensor.matmul(
                            ps1[:],
                            lhsT=wk_t[:, kt, m * P:(m + 1) * P],
                            rhs=kvc_tiles[n1][:, kt, :],
                            start=(kt == 0),
                            stop=(kt == KT_KV - 1),
                        )
                    nc.vector.tensor_copy(khT[m][:, n0 * 512:(n0 + 1) * 512],
                                          ps0[:])
                    nc.vector.tensor_copy(khT[m][:, n1 * 512:(n1 + 1) * 512],
                                          ps1[:])

            def emit_qhT(n):
                nsl = slice(n * 512, (n + 1) * 512)
                qc = qc_tiles[n]
                for m in range(4):  # qhT head-dim tiles
                    ps = psA.tile([P, 512], f32, tag="psA", name="psA")
                    for kt in range(KT_Q):
                        nc.tensor.matmul(
                            ps[:],
                            lhsT=wq_t[:, kt, m * P:(m + 1) * P],
                            rhs=qc[:, kt, :],
                            start=(kt == 0),
                            stop=(kt == KT_Q - 1),
                        )
                    nc.vector.tensor_copy(qhT[m][:, nsl], ps[:])

            # chunk-paced schedule: vh leads (needs only its own chunk), khT
            # consumes chunk pairs, qhT trails.
            emit_vh(0)
            emit_vh(1)
            emit_khT(0)
            emit_vh(2)
            emit_qhT(0)
            emit_vh(3)
            emit_khT(1)
            emit_qhT(1)
            emit_qhT(2)
            emit_qhT(3)

        # ---------------- Phases B+C ----------------
        bc_top = top.enter_context(ExitStack())
        ctxT_pool = bc_top.enter_context(tc.tile_pool(name="ctxT", bufs=1))
        ctxT = [ctxT_pool.tile([P, LQ], dt16, tag=f"ctxT{i}", name=f"ctxT{i}")
                for i in range(4)]

        ps_sum = bc_top.enter_context(tc.tile_pool(name="ps_sum", bufs=1,
                                                   space="PSUM"))
        acc_pool = bc_top.enter_context(tc.tile_pool(name="acc", bufs=2))
        rcb_pool = bc_top.enter_context(tc.tile_pool(name="rcb", bufs=2))
        cx_pool = bc_top.enter_context(tc.tile_pool(name="cx", bufs=4))

        # ---------------- Phase B: attention per head ----------------
        with ExitStack() as ph:
            ps_s = ph.enter_context(tc.tile_pool(name="ps_s", bufs=2,
                                                 space="PSUM"))
            ps_ctx = ph.enter_context(tc.tile_pool(name="ps_ctx", bufs=3,
                                                   space="PSUM"))
            et_pool = ph.enter_context(tc.tile_pool(name="et", bufs=3))
            g_pool = ph.enter_context(tc.tile_pool(name="g", bufs=2))

            scale = 1.0 / np.sqrt(HD)
            pending_tail = [None, None]  # [pss+recip, muls]

            def flush_tail1():
                if pending_tail[0] is not None:
                    pending_tail[0]()
                    pending_tail[0] = None

            def flush_tail():
                flush_tail1()
                if pending_tail[1] is not None:
                    pending_tail[1]()
                    pending_tail[1] = None

            for h in range(2):
                k0, k1 = khT[2 * h], khT[2 * h + 1]
                q0, q1 = qhT[2 * h], qhT[2 * h + 1]
                hsl0 = slice(HD * h, HD * h + P)
                hsl1 = slice(HD * h + P, HD * h + 2 * P)
                for n in range(NQ):
                    nsl = slice(n * 512, (n + 1) * 512)
                    pc0 = ps_ctx.tile([P, 512], f32, tag="pc", name="pc")
                    pc1 = ps_ctx.tile([P, 512], f32, tag="pc", name="pc")
                    g = [None] * 4

                    et_prev = None
                    for kp in range(NKP):
                        kt0, kt1 = 2 * kp, 2 * kp + 1
                        ksl0 = slice(kt0 * P, (kt0 + 1) * P)
                        ksl1 = slice(kt1 * P, (kt1 + 1) * P)
                        # scores for a PAIR of lkv k-tiles into one 2-bank
                        # psum tile; one 1024-wide Exp covers both banks
                        ps2 = ps_s.tile([P, 2, 512], f32, tag="ps_s",
                                        name="ps_s")
                        nc.tensor.matmul(
                            ps2[:, 0, :], lhsT=k0[:, ksl0], rhs=q0[:, nsl],
                            start=True, stop=False,
                        )
                        nc.tensor.matmul(
                            ps2[:, 0, :], lhsT=k1[:, ksl0], rhs=q1[:, nsl],
                            start=False, stop=True,
                        )
                        nc.tensor.matmul(
                            ps2[:, 1, :], lhsT=k0[:, ksl1], rhs=q0[:, nsl],
                            start=True, stop=False,
                        )
                        nc.tensor.matmul(
                            ps2[:, 1, :], lhsT=k1[:, ksl1], rhs=q1[:, nsl],
                            start=False, stop=True,
                        )
                        et2 = et_pool.tile([P, 2, 512], dt16, tag="et",
                                           name="et")
                        nc.scalar.activation(et2[:], ps2[:], Exp, scale=scale,
                                             bias=ebias[:])

                        # sumexp tree accumulation on DVE (all-fp16: 2x rate)
                        j = kp // 2
                        if kp % 2 == 0:
                            g[j] = g_pool.tile([P, 512], dt16, tag=f"g{j}",
                                               name=f"g{j}")
                            nc.vector.tensor_add(g[j][:], et2[:, 0, :],
                                                 et2[:, 1, :])
                        else:
                            gt = g_pool.tile([P, 512], dt16, tag="gt",
                                             name="gt")
                            nc.vector.tensor_add(gt[:], et2[:, 0, :],
                                                 et2[:, 1, :])
                            nc.vector.tensor_add(g[j][:], g[j][:], gt[:])

                        if kp == 1:
                            flush_tail1()
                        elif kp == 2:
                            flush_tail()

                        if et_prev is not None:
                            pkp, pet = et_prev
                            nc.tensor.matmul(
                                pc0[:], lhsT=vh[2 * pkp][:, hsl0],
                                rhs=pet[:, 0, :],
                                start=(pkp == 0), stop=False,
                            )
                            nc.tensor.matmul(
                                pc0[:], lhsT=vh[2 * pkp + 1][:, hsl0],
                                rhs=pet[:, 1, :],
                                start=False, stop=False,
                            )
                            nc.tensor.matmul(
                                pc1[:], lhsT=vh[2 * pkp][:, hsl1],
                                rhs=pet[:, 0, :],
                                start=(pkp == 0), stop=False,
                            )
                            nc.tensor.matmul(
                                pc1[:], lhsT=vh[2 * pkp + 1][:, hsl1],
                                rhs=pet[:, 1, :],
                                start=False, stop=False,
                            )
                        et_prev = (kp, et2)

                    pkp, pet = et_prev
                    nc.tensor.matmul(pc0[:], lhsT=vh[2 * pkp][:, hsl0],
                                     rhs=pet[:, 0, :], start=False, stop=False)
                    nc.tensor.matmul(pc0[:], lhsT=vh[2 * pkp + 1][:, hsl0],
                                     rhs=pet[:, 1, :], start=False, stop=True)
                    nc.tensor.matmul(pc1[:], lhsT=vh[2 * pkp][:, hsl1],
                                     rhs=pet[:, 0, :], start=False, stop=False)
                    nc.tensor.matmul(pc1[:], lhsT=vh[2 * pkp + 1][:, hsl1],
                                     rhs=pet[:, 1, :], start=False, stop=True)

                    # finish the tree: acc = (g0+g1) + (g2+g3), fp16 throughout
                    g01 = g_pool.tile([P, 512], dt16, tag="g01", name="g01")
                    nc.vector.tensor_add(g01[:], g[0][:], g[1][:])
                    g23 = g_pool.tile([P, 512], dt16, tag="g23", name="g23")
                    nc.vector.tensor_add(g23[:], g[2][:], g[3][:])
                    acc = acc_pool.tile([P, 512], dt16, tag="acc", name="acc")
                    nc.vector.tensor_add(acc[:], g01[:], g23[:])

                    # evacuate ctx psum to SBUF now so ps_ctx fits in 3 banks
                    cx0 = cx_pool.tile([P, 512], f32, tag="cx", name="cx")
                    nc.vector.tensor_copy(cx0[:], pc0[:])
                    cx1 = cx_pool.tile([P, 512], f32, tag="cx", name="cx")
                    nc.vector.tensor_copy(cx1[:], pc1[:])

                    def make_tails(cx0=cx0, cx1=cx1, acc=acc, h=h, nsl=nsl):
                        cell = {}

                        def tail1():
                            pss = ps_sum.tile([P, 512], f32, tag="pss",
                                              name="pss")
                            nc.tensor.matmul(pss[:], lhsT=ones_sq[:],
                                             rhs=acc[:], start=True, stop=True)
                            rcb = rcb_pool.tile([P, 512], f32, tag="rcb",
                                                name="rcb")
                            nc.vector.reciprocal(rcb[:], pss[:])
                            cell["rcb"] = rcb

                        def tail2():
                            rcb = cell["rcb"]
                            nc.vector.tensor_mul(ctxT[2 * h][:, nsl], cx0[:],
                                                 rcb[:])
                            nc.vector.tensor_mul(ctxT[2 * h + 1][:, nsl],
                                                 cx1[:], rcb[:])
                        return tail1, tail2

                    pending_tail[0], pending_tail[1] = make_tails()

        # ---------------- Phase C: output projection ----------------
        # n-outer: the last B chunk's normalization (ctxT cols n=3) is flushed
        # up front and has the n=0..2 columns (~20us of matmuls) as runway
        with ExitStack() as ph:
            psC = ph.enter_context(tc.tile_pool(name="psC", bufs=2, space="PSUM"))
            outC = ph.enter_context(tc.tile_pool(name="outC", bufs=4))

            flush_tail()
            for n in range(NQ):  # 4
                nsl = slice(n * 512, (n + 1) * 512)
                for m in range(DQ // P):  # 8
                    ps = psC.tile([P, 512], f32, tag="psC", name="psC")
                    for kt in range(4):
                        nc.tensor.matmul(
                            ps[:],
                            lhsT=wo_t[:, kt, m * P:(m + 1) * P],
                            rhs=ctxT[kt][:, nsl],
                            start=(kt == 0),
                            stop=(kt == 3),
                        )
                    ot = outC.tile([P, 512], dt16, tag="ot", name="ot")
                    nc.scalar.copy(ot[:], ps[:])
                    nc.sync.dma_start(outT[m * P:(m + 1) * P, nsl], ot[:])


def _build():
    import concourse.bacc as bacc
    import concourse.mybir as mybir
    import concourse.tile as tile

    dt16 = mybir.dt.float16
    nc = bacc.Bacc("TRN2", target_bir_lowering=False, debug=False)
    aps = {
        "qPK": nc.dram_tensor("qPK", [P, NQ * KT_Q * 512], dt16,
                              kind="ExternalInput").ap(),
        "kvPK": nc.dram_tensor("kvPK", [P, NQ * KT_KV * 512], dt16,
                               kind="ExternalInput").ap(),
        "wqPK": nc.dram_tensor("wqPK", [P, KT_Q * GH], dt16,
                               kind="ExternalInput").ap(),
        "wkPK": nc.dram_tensor("wkPK", [P, KT_KV * GH], dt16,
                               kind="ExternalInput").ap(),
        "wvPK": nc.dram_tensor("wvPK", [P, KT_KV * GH], dt16,
                               kind="ExternalInput").ap(),
        "woPK": nc.dram_tensor("woPK", [P, 4 * DQ], dt16,
                               kind="ExternalInput").ap(),
        "outT": nc.dram_tensor("outT", [DQ, LQ], dt16,
                               kind="ExternalOutput").ap(),
    }
    with tile.TileContext(nc) as tc:
        _emit(tc, aps)
    nc.compile()
    return nc


def make_in_maps(q, kv, Wq, Wk, Wv, Wo):
    in_maps = []
    for c in range(NCORES):
        b, g = divmod(c, 2)
        hs = slice(g * GH, (g + 1) * GH)
        # pack every tensor into [128, X] rows so each on-device tile is one
        # contiguous 2D DMA block: [p, n, kt, c] etc., flattened along rows
        q_pk = np.ascontiguousarray(
            q[b].reshape(NQ, 512, KT_Q, P).transpose(3, 0, 2, 1)
            .reshape(P, -1), dtype=np.float16)
        kv_pk = np.ascontiguousarray(
            kv[b].reshape(NQ, 512, KT_KV, P).transpose(3, 0, 2, 1)
            .reshape(P, -1), dtype=np.float16)
        wq_pk = np.ascontiguousarray(
            Wq[hs, :].reshape(GH, KT_Q, P).transpose(2, 1, 0)
            .reshape(P, -1), dtype=np.float16)
        wk_pk = np.ascontiguousarray(
            Wk[hs, :].reshape(GH, KT_KV, P).transpose(2, 1, 0)
            .reshape(P, -1), dtype=np.float16)
        wv_pk = np.ascontiguousarray(
            Wv[hs, :].reshape(GH, KT_KV, P).transpose(2, 1, 0)
            .reshape(P, -1), dtype=np.float16)
        wo_pk = np.ascontiguousarray(
            Wo[:, hs].reshape(DQ, 4, P).transpose(2, 1, 0)
            .reshape(P, -1), dtype=np.float16)
        in_maps.append({
            "qPK": q_pk, "kvPK": kv_pk, "wqPK": wq_pk, "wkPK": wk_pk,
            "wvPK": wv_pk, "woPK": wo_pk,
        })
    return in_maps


def kernel(q, kv, Wq, Wk, Wv, Wo, bo):
    global _COMPILED, last_exec_time_ns, last_profile
    from concourse.bass_utils import run_bass_kernel_spmd

    if _COMPILED is None:
        _COMPILED = _build()
    nc = _COMPILED

    q = np.asarray(q, np.float32)
    kv = np.asarray(kv, np.float32)
    Wq = np.asarray(Wq, np.float32)
    Wk = np.asarray(Wk, np.float32)
    Wv = np.asarray(Wv, np.float32)
    Wo = np.asarray(Wo, np.float32)
    bo = np.asarray(bo, np.float32)

    in_maps = make_in_maps(q, kv, Wq, Wk, Wv, Wo)
    res = run_bass_kernel_spmd(nc, in_maps, core_ids=list(range(NCORES)),
                               trace=TRACE)
    last_exec_time_ns = res.exec_time_ns
    last_profile = res.profile_json

    out = np.empty((B, LQ, DQ), np.float32)
    for b in range(B):
        acc = (res.results[2 * b]["outT"].astype(np.float32)
               + res.results[2 * b + 1]["outT"].astype(np.float32))
        out[b] = acc.T + bo
    return out
